# revision 1
# baseline (speedup 1.0000x reference)
"""Multi-head attention (B=2, S=2048, D=1024, H=16, Dk=64) on 8 TRN2 cores.

Sharding: tensor-parallel on heads — 2 heads (dh=128 columns of the QKV
projections) per core.  Each core:
  1. projects qT/kT/vT = (W_slice.T @ x.T) for its 2 heads    [128, 4096]
  2. transposes vT into per-(b,h) [j, d] blocks with an appended
     ones-column (so P@V_aug also yields the softmax row-sums)
  3. scoresT = kT.T-stationary matmul -> pT = exp(scoresT/8) in [j, i]
     layout, PV accumulates oT_aug = [o_unnorm ; rowsums] in PSUM
  4. normalizes via PE-broadcast of 1/rowsum
  5. partialT = Wo_slice.T @ oT                               [1024, 4096]
Host sums the 8 partialT outputs, adds bo, and transposes back.

All matmuls use float32r (full-rate fp32) with fp32 PSUM accumulation.
"""

import numpy as np

D = 1024
NTOK = 4096  # B * S
B = 2
S = 2048
DH = 128  # head-dim block per core (2 heads x 64)
N_CORES = 8

_CACHE = {}


def _build_nc(mm_dtype="float32r"):
    import concourse.bacc as bacc
    import concourse.mybir as mybir
    import concourse.tile as tile

    dt = mybir.dt
    f32 = dt.float32
    mmdt = getattr(dt, mm_dtype)
    AF = mybir.ActivationFunctionType

    def mm(ap):
        return ap

    nc = bacc.Bacc("TRN2", target_bir_lowering=False, debug=False)

    xq = nc.dram_tensor("xq", [D, NTOK], mmdt, kind="ExternalInput").ap()
    xk = nc.dram_tensor("xk", [D, NTOK], mmdt, kind="ExternalInput").ap()
    xv = nc.dram_tensor("xv", [D, NTOK], mmdt, kind="ExternalInput").ap()
    wq = nc.dram_tensor("wq", [128, D], mmdt, kind="ExternalInput").ap()
    wk = nc.dram_tensor("wk", [128, D], mmdt, kind="ExternalInput").ap()
    wv = nc.dram_tensor("wv", [128, D], mmdt, kind="ExternalInput").ap()
    wo = nc.dram_tensor("wo", [128, D], mmdt, kind="ExternalInput").ap()
    bq = nc.dram_tensor("bq", [1, 128], mmdt, kind="ExternalInput").ap()
    bk = nc.dram_tensor("bk", [1, 128], mmdt, kind="ExternalInput").ap()
    bv = nc.dram_tensor("bv", [1, 128], mmdt, kind="ExternalInput").ap()
    c_ident = nc.dram_tensor("c_ident", [128, 64], mmdt, kind="ExternalInput").ap()
    c_ones512 = nc.dram_tensor("c_ones512", [1, 512], mmdt, kind="ExternalInput").ap()
    c_ones64 = nc.dram_tensor("c_ones64", [1, 64], mmdt, kind="ExternalInput").ap()
    pout = nc.dram_tensor("pout", [D, NTOK], f32, kind="ExternalOutput").ap()

    with tile.TileContext(nc) as tc:
        from contextlib import ExitStack

        with ExitStack() as stk:
            const = stk.enter_context(tc.tile_pool(name="const", bufs=1))
            wpool = stk.enter_context(tc.tile_pool(name="w", bufs=1))
            big = stk.enter_context(tc.tile_pool(name="big", bufs=1))
            xpool = stk.enter_context(tc.tile_pool(name="xt", bufs=8))
            ptp = stk.enter_context(tc.tile_pool(name="pt", bufs=4))
            rsp = stk.enter_context(tc.tile_pool(name="rs", bufs=2))
            stp = stk.enter_context(tc.tile_pool(name="st", bufs=4))

            # ---- constants ----
            ident = const.tile([128, 64], mmdt)
            nc.sync.dma_start(out=ident, in_=c_ident)
            ones_row = const.tile([1, 512], mmdt)
            nc.sync.dma_start(out=ones_row, in_=c_ones512)
            ones64 = const.tile([1, 64], mmdt)
            nc.sync.dma_start(out=ones64, in_=c_ones64)

            # ---- weights / biases ----
            wq_sb = wpool.tile([128, D], mmdt)
            wk_sb = wpool.tile([128, D], mmdt)
            wv_sb = wpool.tile([128, D], mmdt)
            wo_sb = wpool.tile([128, D], mmdt)
            nc.sync.dma_start(out=wq_sb, in_=wq)
            nc.sync.dma_start(out=wk_sb, in_=wk)
            nc.sync.dma_start(out=wv_sb, in_=wv)
            nc.sync.dma_start(out=wo_sb, in_=wo)
            bq_sb = const.tile([1, 128], mmdt)
            bk_sb = const.tile([1, 128], mmdt)
            bv_sb = const.tile([1, 128], mmdt)
            nc.sync.dma_start(out=bq_sb, in_=bq)
            nc.sync.dma_start(out=bk_sb, in_=bk)
            nc.sync.dma_start(out=bv_sb, in_=bv)

            # ---- persistent activations ----
            qT = big.tile([128, NTOK], mmdt)  # [dh, tok]
            kT = big.tile([128, NTOK], mmdt)
            vT = big.tile([128, NTOK], mmdt)
            v_sb = big.tile([128, 4 * 16 * 65], mmdt)  # [j, (b,h)*jt*(64+1)]
            oT = big.tile([128, NTOK], mmdt)  # [dh, tok] normalized context

            # ones columns: memset whole tile; v blocks overwritten by transposes
            nc.vector.memset(v_sb, 1.0)
            v_r = v_sb.rearrange("p (t c) -> p t c", c=65)

            def emit_proj(b):
                """projections qT/kT/vT for batch b (cols b*2048..)"""
                with tc.tile_pool(name=f"pp{b}", bufs=8, space="PSUM") as pp:
                    for x_dram, w_sb, b_sb, dst, pnm in (
                        (xq, wq_sb, bq_sb, qT, "q"),
                        (xk, wk_sb, bk_sb, kT, "k"),
                        (xv, wv_sb, bv_sb, vT, "v"),
                    ):
                        acc = [
                            pp.tile([128, 512], f32, tag="pp", name=f"acc{pnm}{b}_{nn}")
                            for nn in range(4)
                        ]
                        for kk in range(8):
                            x_t = xpool.tile([128, 2048], mmdt, tag="xt", name=f"x{pnm}{b}_{kk}")
                            nc.sync.dma_start(
                                out=x_t,
                                in_=x_dram[
                                    kk * 128 : (kk + 1) * 128,
                                    b * 2048 : (b + 1) * 2048,
                                ],
                            )
                            for nn in range(4):
                                nc.tensor.matmul(
                                    acc[nn],
                                    lhsT=w_sb[:, kk * 128 : (kk + 1) * 128],
                                    rhs=x_t[:, nn * 512 : (nn + 1) * 512],
                                    start=(kk == 0),
                                    stop=False,
                                )
                        for nn in range(4):
                            # bias: acc += b_sb.T @ ones  (adds bias to each col)
                            nc.tensor.matmul(
                                acc[nn], lhsT=b_sb, rhs=ones_row, start=False, stop=True
                            )
                        for nn in range(4):
                            col = b * 2048 + nn * 512
                            eng = nc.scalar.copy if nn % 2 == 0 else nc.vector.tensor_copy
                            eng(dst[:, col : col + 512], acc[nn])

            def emit_transp(b):
                """vT -> v_sb [j, d] blocks for batch b"""
                with tc.tile_pool(name=f"tp{b}", bufs=3, space="PSUM") as tpp:
                    for h in range(2):
                        bh = b * 2 + h
                        for g in range(4):  # groups of 4 j-tiles
                            tp = tpp.tile([128, 4 * 64], mmdt, tag="tp", name=f"tp{bh}_{g}")
                            for u in range(4):
                                jb = g * 4 + u
                                nc.tensor.transpose(
                                    tp[:, u * 64 : (u + 1) * 64],
                                    vT[
                                        h * 64 : (h + 1) * 64,
                                        b * 2048 + jb * 128 : b * 2048 + (jb + 1) * 128,
                                    ],
                                    ident[h * 64 : (h + 1) * 64, :],
                                )
                            tp_r = tp.rearrange("p (t c) -> p t c", c=64)
                            nc.scalar.copy(
                                v_r[:, bh * 16 + g * 4 : bh * 16 + g * 4 + 4, 0:64],
                                tp_r,
                            )

            def emit_outproj(b, evac_eng):
                """partialT[:, b cols] = Wo_c.T @ oT ; borrows caller's psum pool"""
                for c4 in range(4):
                    c8 = b * 4 + c4
                    for dt_ in range(8):
                        op = opj_pool[0].tile(
                            [128, 512], f32, tag=opj_pool[1], name=f"op{c8}_{dt_}"
                        )
                        nc.tensor.matmul(
                            op,
                            lhsT=wo_sb[:, dt_ * 128 : (dt_ + 1) * 128],
                            rhs=oT[:, c8 * 512 : (c8 + 1) * 512],
                            start=True,
                            stop=True,
                        )
                        st = stp.tile([128, 512], f32, tag="st", name=f"st{c8}_{dt_}")
                        eng = (
                            nc.vector.tensor_copy
                            if evac_eng == "dve" or dt_ % 2
                            else nc.scalar.copy
                        )
                        eng(st, op)
                        nc.sync.dma_start(
                            out=pout[
                                dt_ * 128 : (dt_ + 1) * 128,
                                c8 * 512 : (c8 + 1) * 512,
                            ],
                            in_=st,
                        )

            # ---- attention passes with deferred finalize ----
            def emit_pass(scp, opp, rpp, b, h, half, pending):
                bh = b * 2 + h
                i0 = b * 2048 + half * 1024
                o_ps = opp.tile([65, 1024], f32, tag="ops", name=f"o{bh}_{half}")
                for jt in range(16):
                    for c in range(2):
                        sc = scp.tile([128, 512], f32, tag="sc", name=f"s{bh}_{half}_{jt}_{c}")
                        nc.tensor.matmul(
                            sc,
                            lhsT=kT[
                                h * 64 : (h + 1) * 64,
                                b * 2048 + jt * 128 : b * 2048 + (jt + 1) * 128,
                            ],
                            rhs=qT[h * 64 : (h + 1) * 64, i0 + c * 512 : i0 + (c + 1) * 512],
                            start=True,
                            stop=True,
                        )
                        pt = ptp.tile([128, 512], mmdt, tag="pt", name=f"p{bh}_{half}_{jt}_{c}")
                        nc.scalar.activation(pt, sc, AF.Exp, scale=0.125)
                        nc.tensor.matmul(
                            o_ps[:, c * 512 : (c + 1) * 512],
                            lhsT=v_sb[:, (bh * 16 + jt) * 65 : (bh * 16 + jt + 1) * 65],
                            rhs=pt,
                            start=(jt == 0),
                            stop=(jt == 15),
                        )
                    if jt == 2 and pending is not None:
                        emit_finalize(rpp, *pending)
                        pending = None
                return (o_ps, b, h, half)

            def emit_finalize(rpp, o_ps, b, h, half):
                """normalize: oT[h cols] = o_unnorm * broadcast(1/rowsum)"""
                bh = b * 2 + h
                i0 = b * 2048 + half * 1024
                rinv = rsp.tile([1, 1024], mmdt, tag="rinv", name=f"ri{bh}_{half}")
                with nc.allow_low_precision(reason="fp16 rinv is plenty"):
                    nc.vector.reciprocal(rinv, o_ps[64:65, :])
                Rp = rpp.tile([64, 1024], f32, tag="rp", name=f"R{bh}_{half}")
                for c in range(2):
                    nc.tensor.matmul(
                        Rp[:, c * 512 : (c + 1) * 512],
                        lhsT=ones64,
                        rhs=rinv[:, c * 512 : (c + 1) * 512],
                        start=True,
                        stop=True,
                    )
                Rs = rsp.tile([64, 1024], f32, tag="rs", name=f"Rs{bh}_{half}")
                nc.vector.tensor_copy(Rs, Rp)
                nc.vector.tensor_mul(
                    oT[h * 64 : (h + 1) * 64, i0 : i0 + 1024], o_ps[0:64, :], Rs
                )

            # =========== emission schedule ===========
            emit_proj(0)
            emit_transp(0)
            pending = None
            with (
                tc.tile_pool(name="scA", bufs=2, space="PSUM") as scA,
                tc.tile_pool(name="opsA", bufs=2, space="PSUM") as opsA,
                tc.tile_pool(name="rpA", bufs=1, space="PSUM") as rpA,
            ):
                for h in range(2):
                    for half in range(2):
                        pending = emit_pass(scA, opsA, rpA, 0, h, half, pending)
                emit_finalize(rpA, *pending)
                pending = None

            emit_proj(1)
            emit_transp(1)
            with (
                tc.tile_pool(name="scB", bufs=2, space="PSUM") as scB,
                tc.tile_pool(name="opsB", bufs=2, space="PSUM") as opsB,
                tc.tile_pool(name="rpB", bufs=1, space="PSUM") as rpB,
            ):
                pending = emit_pass(scB, opsB, rpB, 1, 0, 0, pending)
                pending = emit_pass(scB, opsB, rpB, 1, 0, 1, pending)
                # b0 out-projection overlaps b1 attention (borrows scB slots)
                opj_pool = (scB, "sc")
                emit_outproj(0, "dve")
                pending = emit_pass(scB, opsB, rpB, 1, 1, 0, pending)
                pending = emit_pass(scB, opsB, rpB, 1, 1, 1, pending)
                emit_finalize(rpB, *pending)
                opj_pool = (scB, "sc")
                emit_outproj(1, "mix")

    nc.compile()
    return nc


MM_DTYPE = "float16"


def _get_nc():
    key = ("nc", MM_DTYPE)
    if key not in _CACHE:
        _CACHE[key] = _build_nc(MM_DTYPE)
    return _CACHE[key]


def _ensure_ntff_hook():
    """Register the NTFF profile hook module if the image lacks it."""
    import sys
    import types

    if "antenv.axon_hooks" in sys.modules:
        return
    try:
        from trn_agent_boot.trn_boot import _ntff_profile_via_ctypes
    except Exception:
        return
    hook = None
    try:
        hook = _ntff_profile_via_ctypes("/opt/axon/libaxon_pjrt.so")
    except Exception:
        hook = None
    mod = types.ModuleType("antenv.axon_hooks")
    mod._hook = hook
    mod.get_axon_ntff_profile_hook = lambda: mod._hook
    mod.set_axon_ntff_profile_hook = lambda h: setattr(mod, "_hook", h)
    sys.modules["antenv.axon_hooks"] = mod


def _run(inputs, trace=False):
    from concourse import bass_utils

    if trace:
        _ensure_ntff_hook()

    nc = _get_nc()
    query = np.asarray(inputs["query"], np.float32)
    key = np.asarray(inputs["key"], np.float32)
    value = np.asarray(inputs["value"], np.float32)
    Wq = np.asarray(inputs["Wq"], np.float32)
    Wk = np.asarray(inputs["Wk"], np.float32)
    Wv = np.asarray(inputs["Wv"], np.float32)
    Wo = np.asarray(inputs["Wo"], np.float32)
    bq = np.asarray(inputs["bq"], np.float32)
    bk = np.asarray(inputs["bk"], np.float32)
    bv = np.asarray(inputs["bv"], np.float32)
    bo = np.asarray(inputs["bo"], np.float32)

    if MM_DTYPE == "bfloat16":
        import ml_dtypes

        ext_dt = ml_dtypes.bfloat16
    elif MM_DTYPE == "float16":
        ext_dt = np.float16
    else:
        ext_dt = np.float32

    xqT = np.ascontiguousarray(query.reshape(NTOK, D).T.astype(ext_dt))
    xkT = np.ascontiguousarray(key.reshape(NTOK, D).T.astype(ext_dt))
    xvT = np.ascontiguousarray(value.reshape(NTOK, D).T.astype(ext_dt))

    def pack_w(Wc):
        return np.ascontiguousarray(
            Wc.reshape(8, 128, 128).transpose(1, 0, 2).reshape(128, D).astype(ext_dt)
        )

    ident_np = np.zeros((128, 64), np.float32)
    ident_np[np.arange(64), np.arange(64)] = 1.0
    ident_np[64 + np.arange(64), np.arange(64)] = 1.0
    consts = {
        "c_ident": np.ascontiguousarray(ident_np.astype(ext_dt)),
        "c_ones512": np.ones((1, 512), ext_dt),
        "c_ones64": np.ones((1, 64), ext_dt),
    }
    in_maps = []
    for c in range(N_CORES):
        sl = slice(c * 128, (c + 1) * 128)
        in_maps.append(
            {
                **consts,
                "xq": xqT,
                "xk": xkT,
                "xv": xvT,
                "wq": pack_w(Wq[:, sl]),
                "wk": pack_w(Wk[:, sl]),
                "wv": pack_w(Wv[:, sl]),
                "wo": np.ascontiguousarray(Wo[sl, :].astype(ext_dt)),
                "bq": np.ascontiguousarray(bq[sl].reshape(1, 128).astype(ext_dt)),
                "bk": np.ascontiguousarray(bk[sl].reshape(1, 128).astype(ext_dt)),
                "bv": np.ascontiguousarray(bv[sl].reshape(1, 128).astype(ext_dt)),
            }
        )

    res = bass_utils.run_bass_kernel_spmd(
        nc, in_maps, core_ids=list(range(N_CORES)), trace=trace
    )
    outT = np.zeros((D, NTOK), np.float64)
    for c in range(N_CORES):
        outT += np.asarray(res.results[c]["pout"], np.float64)
    out = (outT.T + bo.astype(np.float64)).astype(np.float32)
    return out.reshape(B, S, D), res


def kernel(**inputs):
    out, _ = _run(inputs, trace=False)
    return out



# revision 2
# speedup vs baseline: 1.3999x; 1.3999x over previous
"""Multi-head attention (B=2, S=2048, D=1024, H=16, Dk=64) on 8 TRN2 cores.

Sharding: batch-split x head-TP.  Core c handles batch c//4 and heads
hs*4..hs*4+3 where hs = c%4 (256 projection dims = 2 "ob" blocks of 128).
Each core:
  1. projects kT/vT/qT = (W_slice.T @ x.T) for its 4 heads   [2x[128, 2048]]
  2. transposes vT into per-(ob,h) [j, d] blocks with an appended
     ones-column (so P@V_aug also yields the softmax row-sums)
  3. pipelined attention per (ob, half): scoresT -> exp (FD=1024 ACT)
     -> PV accumulate [65, 1024] PSUM; the 1/rowsum PE-broadcast lands in
     partitions 64:128 of the same PSUM banks.
  4. partialT = Wo_slice.T @ oT  (K=256 accumulated over both obs)
Host sums 4 partials per batch, adds bo, transposes back.

All matmuls fp16 operands with fp32 PSUM accumulation.
"""

import numpy as np

D = 1024
S = 2048  # tokens per batch (= per core)
B = 2
N_CORES = 8

_CACHE = {}


def _build_nc(mm_dtype="float16"):
    import concourse.bacc as bacc
    import concourse.mybir as mybir
    import concourse.tile as tile

    dt = mybir.dt
    f32 = dt.float32
    mmdt = getattr(dt, mm_dtype)
    AF = mybir.ActivationFunctionType

    nc = bacc.Bacc("TRN2", target_bir_lowering=False, debug=False)

    xq = nc.dram_tensor("xq", [D, S], mmdt, kind="ExternalInput").ap()
    xk = nc.dram_tensor("xk", [D, S], mmdt, kind="ExternalInput").ap()
    xv = nc.dram_tensor("xv", [D, S], mmdt, kind="ExternalInput").ap()
    wq = nc.dram_tensor("wq", [128, 2048], mmdt, kind="ExternalInput").ap()
    wk = nc.dram_tensor("wk", [128, 2048], mmdt, kind="ExternalInput").ap()
    wv = nc.dram_tensor("wv", [128, 2048], mmdt, kind="ExternalInput").ap()
    wo = nc.dram_tensor("wo", [128, 2048], mmdt, kind="ExternalInput").ap()
    bias6 = nc.dram_tensor("bias6", [128, 6], f32, kind="ExternalInput").ap()
    c_ident = nc.dram_tensor("c_ident", [128, 64], mmdt, kind="ExternalInput").ap()
    c_ones64 = nc.dram_tensor("c_ones64", [1, 64], mmdt, kind="ExternalInput").ap()
    pout = nc.dram_tensor("pout", [D, S], mmdt, kind="ExternalOutput").ap()

    with tile.TileContext(nc) as tc:
        from contextlib import ExitStack

        with ExitStack() as stk:
            const = stk.enter_context(tc.tile_pool(name="const", bufs=1))
            wpool = stk.enter_context(tc.tile_pool(name="w", bufs=1))
            big = stk.enter_context(tc.tile_pool(name="big", bufs=1))
            xpool = stk.enter_context(tc.tile_pool(name="xt", bufs=6))
            ptp = stk.enter_context(tc.tile_pool(name="pt", bufs=4))
            rsp = stk.enter_context(tc.tile_pool(name="rs", bufs=2))
            stp = stk.enter_context(tc.tile_pool(name="st", bufs=4))

            # ---- constants ----
            ident = const.tile([128, 64], mmdt)
            nc.sync.dma_start(out=ident, in_=c_ident)
            ones64 = const.tile([1, 64], mmdt)
            nc.sync.dma_start(out=ones64, in_=c_ones64)
            bias_sb = const.tile([128, 6], f32)
            nc.sync.dma_start(out=bias_sb, in_=bias6)

            # ---- weights ----
            wq_sb = wpool.tile([128, 2048], mmdt)
            wk_sb = wpool.tile([128, 2048], mmdt)
            wv_sb = wpool.tile([128, 2048], mmdt)
            wo_sb = wpool.tile([128, 2048], mmdt)
            nc.sync.dma_start(out=wk_sb, in_=wk)
            nc.sync.dma_start(out=wv_sb, in_=wv)
            nc.sync.dma_start(out=wq_sb, in_=wq)
            nc.sync.dma_start(out=wo_sb, in_=wo)

            # ---- persistent activations ----
            qT2 = big.tile([128, 4096], mmdt)  # [dh within ob, ob*2048 + tok]
            kT2 = big.tile([128, 4096], mmdt)
            vT2 = big.tile([128, 4096], mmdt)
            oT2 = big.tile([128, 4096], mmdt)
            v_sb = big.tile([128, 4 * 16 * 65], mmdt)  # [j, (ob,h)*jt*(64+1)]
            nc.vector.memset(v_sb, 1.0)
            v_r = v_sb.rearrange("p (t c) -> p t c", c=65)

            def emit_proj(x_dram, w_sb, dst, bias_col0, pnm):
                """dst[:, ob*2048 + tok] = W.T @ x + b for both ob blocks."""
                with tc.tile_pool(name=f"pp{pnm}", bufs=8, space="PSUM") as pp:
                    acc = [
                        pp.tile([128, 512], f32, tag="pp", name=f"acc{pnm}_{a}")
                        for a in range(8)
                    ]
                    for kk in range(8):
                        x_t = xpool.tile([128, 2048], mmdt, tag="xt", name=f"x{pnm}{kk}")
                        nc.sync.dma_start(
                            out=x_t, in_=x_dram[kk * 128 : (kk + 1) * 128, :]
                        )
                        for ob in range(2):
                            for n in range(4):
                                nc.tensor.matmul(
                                    acc[ob * 4 + n],
                                    lhsT=w_sb[:, (kk * 2 + ob) * 128 : (kk * 2 + ob + 1) * 128],
                                    rhs=x_t[:, n * 512 : (n + 1) * 512],
                                    start=(kk == 0),
                                    stop=(kk == 7),
                                )
                    for ob in range(2):
                        for n in range(4):
                            nc.vector.tensor_scalar_add(
                                dst[:, ob * 2048 + n * 512 : ob * 2048 + (n + 1) * 512],
                                acc[ob * 4 + n],
                                bias_sb[:, bias_col0 + ob : bias_col0 + ob + 1],
                            )

            def emit_transp():
                """vT2 -> v_sb [j, d] blocks for all 4 head-slots."""
                with tc.tile_pool(name="tp", bufs=3, space="PSUM") as tpp:
                    for ob in range(2):
                        for h in range(2):
                            bh = ob * 2 + h
                            for g in range(4):
                                tp = tpp.tile(
                                    [128, 4 * 64], mmdt, tag="tp", name=f"tp{bh}_{g}"
                                )
                                for u in range(4):
                                    jb = g * 4 + u
                                    nc.tensor.transpose(
                                        tp[:, u * 64 : (u + 1) * 64],
                                        vT2[
                                            h * 64 : (h + 1) * 64,
                                            ob * 2048 + jb * 128 : ob * 2048 + (jb + 1) * 128,
                                        ],
                                        ident[h * 64 : (h + 1) * 64, :],
                                    )
                                tp_r = tp.rearrange("p (t c) -> p t c", c=64)
                                nc.scalar.copy(
                                    v_r[:, bh * 16 + g * 4 : bh * 16 + g * 4 + 4, 0:64],
                                    tp_r,
                                )

            # =========== emission schedule ===========
            emit_proj(xk, wk_sb, kT2, 2, "k")
            emit_proj(xv, wv_sb, vT2, 4, "v")
            emit_transp()
            emit_proj(xq, wq_sb, qT2, 0, "q")

            # ---- attention: pipelined over (ob, half, jt) ----
            def emit_finalize(o_ps, i0, tag):
                for h in range(2):
                    rinv = rsp.tile([1, 1024], mmdt, tag="ri", name=f"ri{tag}_{h}")
                    with nc.allow_low_precision(reason="fp16 rinv is plenty"):
                        nc.vector.reciprocal(rinv, o_ps[h][64:65, :])
                    for c in range(2):
                        nc.tensor.matmul(
                            o_ps[h][64:128, c * 512 : (c + 1) * 512],
                            lhsT=ones64,
                            rhs=rinv[:, c * 512 : (c + 1) * 512],
                            start=True,
                            stop=True,
                        )
                    Rs = rsp.tile([64, 1024], f32, tag="rs", name=f"Rs{tag}_{h}")
                    nc.vector.tensor_copy(Rs, o_ps[h][64:128, :])
                    nc.vector.tensor_mul(
                        oT2[h * 64 : (h + 1) * 64, i0 : i0 + 1024],
                        o_ps[h][0:64, :],
                        Rs,
                    )

            with (
                tc.tile_pool(name="scp", bufs=2, space="PSUM") as scp,
                tc.tile_pool(name="opp", bufs=2, space="PSUM") as opp,
            ):
                pending = None
                for ob in range(2):
                    for half in range(2):
                        i0 = ob * 2048 + half * 1024
                        o_ps = [
                            opp.tile([128, 1024], f32, tag="ops", name=f"o{ob}_{half}_{h}")
                            for h in range(2)
                        ]
                        pt_prev = [None, None]
                        for jt in range(17):
                            for h in range(2):
                                if jt < 16:
                                    sc = scp.tile(
                                        [128, 1024], f32, tag="sc",
                                        name=f"s{ob}_{half}_{jt}_{h}",
                                    )
                                    for c in range(2):
                                        nc.tensor.matmul(
                                            sc[:, c * 512 : (c + 1) * 512],
                                            lhsT=kT2[
                                                h * 64 : (h + 1) * 64,
                                                ob * 2048 + jt * 128 : ob * 2048 + (jt + 1) * 128,
                                            ],
                                            rhs=qT2[
                                                h * 64 : (h + 1) * 64,
                                                i0 + c * 512 : i0 + (c + 1) * 512,
                                            ],
                                            start=True,
                                            stop=True,
                                        )
                                    pt = ptp.tile(
                                        [128, 1024], mmdt, tag="pt",
                                        name=f"p{ob}_{half}_{jt}_{h}",
                                    )
                                    nc.scalar.activation(pt, sc, AF.Exp, scale=0.125)
                                if jt > 0:
                                    jp = jt - 1
                                    bh = ob * 2 + h
                                    for c in range(2):
                                        nc.tensor.matmul(
                                            o_ps[h][0:65, c * 512 : (c + 1) * 512],
                                            lhsT=v_sb[
                                                :, (bh * 16 + jp) * 65 : (bh * 16 + jp + 1) * 65
                                            ],
                                            rhs=pt_prev[h][:, c * 512 : (c + 1) * 512],
                                            start=(jp == 0),
                                            stop=(jp == 15),
                                        )
                                if jt < 16:
                                    pt_prev[h] = pt
                            # deferred finalize of the previous (ob, half):
                            # emitted after this half's prologue is in flight
                            if jt == 1 and pending is not None:
                                emit_finalize(*pending)
                                pending = None
                        pending = (o_ps, i0, f"{ob}_{half}")
                emit_finalize(*pending)

            # ---- out-projection: partial = Wo_slice.T @ oT (K=256) ----
            with tc.tile_pool(name="opj", bufs=4, space="PSUM") as pj:
                for dtb in range(8):
                    ops = [
                        pj.tile([128, 512], f32, tag="pj", name=f"pj{dtb}_{c}")
                        for c in range(4)
                    ]
                    for ob in range(2):
                        for c in range(4):
                            nc.tensor.matmul(
                                ops[c],
                                lhsT=wo_sb[:, (ob * 8 + dtb) * 128 : (ob * 8 + dtb + 1) * 128],
                                rhs=oT2[:, ob * 2048 + c * 512 : ob * 2048 + (c + 1) * 512],
                                start=(ob == 0),
                                stop=(ob == 1),
                            )
                    for c in range(4):
                        st = stp.tile([128, 512], mmdt, tag="st", name=f"st{dtb}_{c}")
                        eng = nc.vector.tensor_copy if c % 2 else nc.scalar.copy
                        eng(st, ops[c])
                        nc.sync.dma_start(
                            out=pout[
                                dtb * 128 : (dtb + 1) * 128, c * 512 : (c + 1) * 512
                            ],
                            in_=st,
                        )

    nc.compile()
    return nc


MM_DTYPE = "float16"


def _get_nc():
    key = ("nc", MM_DTYPE)
    if key not in _CACHE:
        _CACHE[key] = _build_nc(MM_DTYPE)
    return _CACHE[key]


def _ensure_ntff_hook():
    """Register the NTFF profile hook module if the image lacks it."""
    import sys
    import types

    if "antenv.axon_hooks" in sys.modules:
        return
    try:
        from trn_agent_boot.trn_boot import _ntff_profile_via_ctypes
    except Exception:
        return
    hook = None
    try:
        hook = _ntff_profile_via_ctypes("/opt/axon/libaxon_pjrt.so")
    except Exception:
        hook = None
    mod = types.ModuleType("antenv.axon_hooks")
    mod._hook = hook
    mod.get_axon_ntff_profile_hook = lambda: mod._hook
    mod.set_axon_ntff_profile_hook = lambda h: setattr(mod, "_hook", h)
    sys.modules["antenv.axon_hooks"] = mod


def _make_in_maps(inputs, ext_dt):
    query = np.asarray(inputs["query"], np.float32)
    key = np.asarray(inputs["key"], np.float32)
    value = np.asarray(inputs["value"], np.float32)
    Wq = np.asarray(inputs["Wq"], np.float32)
    Wk = np.asarray(inputs["Wk"], np.float32)
    Wv = np.asarray(inputs["Wv"], np.float32)
    Wo = np.asarray(inputs["Wo"], np.float32)
    bq = np.asarray(inputs["bq"], np.float32)
    bk = np.asarray(inputs["bk"], np.float32)
    bv = np.asarray(inputs["bv"], np.float32)

    # per-batch transposed inputs [D, S]
    xT = {}
    for b in range(B):
        xT[("q", b)] = np.ascontiguousarray(query[b].T.astype(ext_dt))
        xT[("k", b)] = np.ascontiguousarray(key[b].T.astype(ext_dt))
        xT[("v", b)] = np.ascontiguousarray(value[b].T.astype(ext_dt))

    ident_np = np.zeros((128, 64), np.float32)
    ident_np[np.arange(64), np.arange(64)] = 1.0
    ident_np[64 + np.arange(64), np.arange(64)] = 1.0
    consts = {
        "c_ident": np.ascontiguousarray(ident_np.astype(ext_dt)),
        "c_ones64": np.ones((1, 64), ext_dt),
    }

    def pack_w(Wc):  # [1024, 256] -> [128, 2048] as (kk, ob) tiles
        return np.ascontiguousarray(
            Wc.reshape(8, 128, 2, 128).transpose(1, 0, 2, 3).reshape(128, 2048).astype(ext_dt)
        )

    def pack_wo(Wc):  # [256, 1024] -> [128, 2048] as (ob, dt) tiles
        return np.ascontiguousarray(
            Wc.reshape(2, 128, 8, 128).transpose(1, 0, 2, 3).reshape(128, 2048).astype(ext_dt)
        )

    in_maps = []
    for c in range(N_CORES):
        b, hs = divmod(c, 4)
        sl = slice(hs * 256, (hs + 1) * 256)
        bias6 = np.zeros((128, 6), np.float32)
        bias6[:, 0] = bq[sl][0:128]
        bias6[:, 1] = bq[sl][128:256]
        bias6[:, 2] = bk[sl][0:128]
        bias6[:, 3] = bk[sl][128:256]
        bias6[:, 4] = bv[sl][0:128]
        bias6[:, 5] = bv[sl][128:256]
        in_maps.append(
            {
                **consts,
                "xq": xT[("q", b)],
                "xk": xT[("k", b)],
                "xv": xT[("v", b)],
                "wq": pack_w(Wq[:, sl]),
                "wk": pack_w(Wk[:, sl]),
                "wv": pack_w(Wv[:, sl]),
                "wo": pack_wo(Wo[sl, :]),
                "bias6": np.ascontiguousarray(bias6),
            }
        )
    return in_maps


def _gather(results, bo):
    outT = np.zeros((B, D, S), np.float64)
    for c in range(N_CORES):
        outT[c // 4] += np.asarray(results[c]["pout"], np.float64)
    out = outT.transpose(0, 2, 1) + bo.astype(np.float64)
    return out.astype(np.float32)


def _run(inputs, trace=False):
    from concourse import bass_utils

    if trace:
        _ensure_ntff_hook()

    nc = _get_nc()
    if MM_DTYPE == "bfloat16":
        import ml_dtypes

        ext_dt = ml_dtypes.bfloat16
    elif MM_DTYPE == "float16":
        ext_dt = np.float16
    else:
        ext_dt = np.float32

    in_maps = _make_in_maps(inputs, ext_dt)
    res = bass_utils.run_bass_kernel_spmd(
        nc, in_maps, core_ids=list(range(N_CORES)), trace=trace
    )
    bo = np.asarray(inputs["bo"], np.float32)
    out = _gather(res.results, bo)
    return out.reshape(B, S, D), res


def kernel(**inputs):
    out, _ = _run(inputs, trace=False)
    return out
